# revision 2
# baseline (speedup 1.0000x reference)
"""DiffusionGraphConv on 8 Trainium2 NeuronCores (Bass/Tile), v2.

out = sum_k (D^-1 A)^k x W_f[k] + ((D^-1 A)^T)^k x W_b[k] + bias, K=2,
N=50000 nodes, E=800000 edges, B=8, C_in=C_out=64, f32.

Sharding: 8 cores = 2 batch-quads x 2 directions x 2 dst-halves.
Tokens are 512B (4 batches x 64 feats, bf16) so each DMA-gather descriptor
runs at full bus bandwidth.  The pair (2k, 2k+1) = one (quad, direction)
job split by destination blocks (half A: blocks 0..195, half B: 196..390);
h1 = A@h0 is exchanged through a pair-shared HBM tensor: scatter-add into
a zeroed buffer with data-driven row indices keeps the SPMD instruction
stream identical across cores (each scatter call is entirely-valid or
entirely-dead per core; num_idxs_reg comes from input data via reg_load).
A tiny pairwise AllGather is the inter-hop barrier.

Per hop, per dst slot: one-hot scatter matrices S'[t,r] are built by one
DVE tensor_scalar (bf16) and the sparse accumulate is a PE matmul per
128-edge chunk into PSUM.  Hop 2 computes h2 transposed (gathered chunk as
the stationary operand) so the output weight matmuls need no PE
transposes; the W0 term reuses h1 via a transpose-mode dma_gather from a
local slot-major copy.
"""
import numpy as np
import ml_dtypes

import concourse.bacc as bacc
import concourse.tile as tile
import concourse.mybir as mybir
from concourse.bass_utils import run_bass_kernel_spmd

P = 128
N_NODES = 50000
N_EDGES = 800000
B, C = 8, 64
F = 256                  # feats per token: 4 batches x 64, bf16 -> 512B
NNP = 50048              # nodes padded to a multiple of 128
NB = NNP // P            # 391 blocks
NSLOT = 196              # slots per half (A: blocks 0..195, B: 196..390+dummy)
NA = 196                 # blocks in half A
LO_LIMIT = 32768         # lo gather window [0, 32768)
HI_BASE = NNP - 32768    # hi window [17280, NNP)
SH_ROWS = NNP + P        # shared h1 + pad rows (barrier dep marker)
H1L_ROWS = NSLOT * P     # local slot-major h1 copy
GS = 2048                # gather slab tokens
ISLAB = 8192             # idx slab tokens (4 gather slabs per idx dma)
SCB = 16                 # slots per scatter/staging batch
NBATCH = (NSLOT + SCB - 1) // SCB
LASTB = NSLOT - (NBATCH - 1) * SCB      # slots in the last batch (4)
bf16 = ml_dtypes.bfloat16
dt = mybir.dt

_prog_cache = {}
_prep_cache = {}


# ---------------- host-side prep ----------------

def _variant_counts(dst, src, blocks):
    """(must_lo, must_hi, flex, tot) per slot for one (direction, half)."""
    blk = dst >> 7
    inv = np.full(NB, -1, np.int64)
    inv[blocks] = np.arange(len(blocks))
    m = inv[blk] >= 0
    slot = inv[blk[m]]
    s = src[m]
    must_lo = np.bincount(slot[s < HI_BASE], minlength=NSLOT)
    must_hi = np.bincount(slot[s >= LO_LIMIT], minlength=NSLOT)
    flex = np.bincount(slot[(s >= HI_BASE) & (s < LO_LIMIT)], minlength=NSLOT)
    return must_lo, must_hi, flex, must_lo + must_hi + flex


def _choose_chunks4(cnts):
    """Shared per-slot (L, H) minimizing L+H over the 4 stream variants."""
    L = np.zeros(NSLOT, np.int64)
    H = np.zeros(NSLOT, np.int64)
    f2l = [np.zeros(NSLOT, np.int64) for _ in cnts]
    for j in range(NSLOT):
        lmin = max((ml[j] + P - 1) // P for ml, mh, fx, tot in cnts)
        best = None
        for Lc in range(lmin, lmin + 4):
            Hc = 0
            for ml, mh, fx, tot in cnts:
                lo = min(Lc * P, ml[j] + fx[j])
                Hc = max(Hc, (max(mh[j], tot[j] - lo) + P - 1) // P)
            if best is None or Lc + Hc < best[0] + best[1]:
                best = (Lc, Hc)
        L[j], H[j] = best
        for v, (ml, mh, fx, tot) in enumerate(cnts):
            lo = min(L[j] * P, ml[j] + fx[j])
            lo = max(lo, tot[j] - H[j] * P)   # ensure hi fits
            f2l[v][j] = lo - ml[j]
    return L, H, f2l


def _build_stream(dst, src, nv, blocks, L, H, flex_to_lo):
    """Padded lo/hi token streams + chunk-major meta for one variant."""
    lo_chunk_off = np.concatenate([[0], np.cumsum(L)[:-1]])
    hi_chunk_off = np.concatenate([[0], np.cumsum(H)[:-1]])
    chunk_off = np.concatenate([[0], np.cumsum(L + H)[:-1]])
    NCH = int((L + H).sum())
    TLO, THI = int(L.sum()) * P, int(H.sum()) * P

    blk = (dst >> 7).astype(np.int64)
    inv = np.full(NB, -1, np.int64)
    inv[blocks] = np.arange(len(blocks))
    m = inv[blk] >= 0
    d_s, s_s, nv_s = dst[m], src[m], nv[m]
    slot = inv[blk[m]]

    lo = s_s < HI_BASE
    flex = (s_s >= HI_BASE) & (s_s < LO_LIMIT)
    fidx = np.flatnonzero(flex)
    forder = np.argsort(slot[fidx], kind="stable")
    fslot = slot[fidx[forder]]
    fcnt = np.bincount(fslot, minlength=NSLOT)
    fstart = np.concatenate([[0], np.cumsum(fcnt)[:-1]])
    frank = np.arange(fidx.size) - fstart[fslot]
    lo = lo.copy()
    lo[fidx[forder]] = frank < flex_to_lo[fslot]
    assert (np.bincount(slot[lo], minlength=NSLOT) <= L * P).all()
    assert (np.bincount(slot[~lo], minlength=NSLOT) <= H * P).all()

    order = np.lexsort((~lo, slot))
    d_s, s_s, nv_s = d_s[order], s_s[order], nv_s[order]
    slot_s, lo_s = slot[order], lo[order]
    gid = slot_s * 2 + (~lo_s).astype(np.int64)
    cnt = np.bincount(gid, minlength=NSLOT * 2)
    gstart = np.concatenate([[0], np.cumsum(cnt)[:-1]])
    rank = np.arange(d_s.size) - gstart[gid]
    pos = np.where(lo_s, lo_chunk_off[slot_s] * P + rank,
                   hi_chunk_off[slot_s] * P + rank)

    idx_lo = np.zeros(TLO, np.int16)
    nv_lo = np.zeros(TLO, np.float32)
    rm_lo = np.zeros(TLO, np.float32)
    idx_hi = np.zeros(THI, np.int16)
    nv_hi = np.zeros(THI, np.float32)
    rm_hi = np.zeros(THI, np.float32)
    mm = lo_s
    idx_lo[pos[mm]] = s_s[mm].astype(np.int16)
    nv_lo[pos[mm]] = nv_s[mm]
    rm_lo[pos[mm]] = (d_s[mm] & 127).astype(np.float32)
    mm = ~lo_s
    idx_hi[pos[mm]] = (s_s[mm] - HI_BASE).astype(np.int16)
    nv_hi[pos[mm]] = nv_s[mm]
    rm_hi[pos[mm]] = (d_s[mm] & 127).astype(np.float32)

    rowm = np.zeros((P, NCH), np.float32)
    nvm = np.zeros((P, NCH), np.float32)
    lo_cols = np.concatenate(
        [chunk_off[j] + np.arange(L[j]) for j in range(NSLOT)])
    hi_cols = np.concatenate(
        [chunk_off[j] + L[j] + np.arange(H[j]) for j in range(NSLOT)])
    if TLO:
        rowm[:, lo_cols] = rm_lo.reshape(-1, P).T
        nvm[:, lo_cols] = nv_lo.reshape(-1, P).T
    if THI:
        rowm[:, hi_cols] = rm_hi.reshape(-1, P).T
        nvm[:, hi_cols] = nv_hi.reshape(-1, P).T
    return idx_lo, idx_hi, rowm, nvm


def _wrap16(a):
    """[T] -> [128, T/16]; token i at [i%16, i//16], replicated 8x."""
    return np.ascontiguousarray(np.tile(a.reshape(-1, 16).T, (8, 1)))


def _scatter_idx(blocks):
    """Scatter row idx streams, split by half OWNERSHIP (pure per core):
    half A rows (< 25088) go to the lo call, half B rows to the hi call."""
    rows = np.full(NSLOT * P, -1, np.int64)
    for j, b in enumerate(blocks):
        rows[j * P:(j + 1) * P] = b * P + np.arange(P)
    dummy = slice(len(blocks) * P, NSLOT * P)
    rows[dummy] = NNP - 1   # always-written row; dummy h1 value is 0
    lo = np.where((rows >= 0) & (rows < NA * P), rows, -1).astype(np.int16)
    hi = np.where(rows >= NA * P, rows - HI_BASE, -1).astype(np.int16)
    return _wrap16(lo), _wrap16(hi)


def _prep(edge_index, edge_vals):
    rows = edge_index[0].astype(np.int64)
    cols = edge_index[1].astype(np.int64)
    deg = np.zeros(N_NODES, np.float32)
    np.add.at(deg, rows, edge_vals)
    deg += np.float32(1e-8)
    nv = (edge_vals / deg[rows]).astype(np.float32)

    # half split + rank-matched slot pairing
    cntf = np.bincount(rows >> 7, minlength=NB)  # fwd: dst=rows
    cntb = np.bincount(cols >> 7, minlength=NB)  # bwd: dst=cols
    key = np.maximum(cntf, cntb)
    blocks_a = np.arange(0, NA)
    blocks_b = np.arange(NA, NB)
    map_a = blocks_a[np.argsort(-key[blocks_a], kind="stable")]
    map_b = blocks_b[np.argsort(-key[blocks_b], kind="stable")]

    variants = [(rows, cols, map_a), (rows, cols, map_b),
                (cols, rows, map_a), (cols, rows, map_b)]
    cnts = [_variant_counts(d, s, m) for d, s, m in variants]
    L, H, f2l = _choose_chunks4(cnts)
    streams = [_build_stream(d, s, nv, m, L, H, f2l[v])
               for v, (d, s, m) in enumerate(variants)]
    scat = {0: _scatter_idx(map_a), 1: _scatter_idx(map_b)}
    h1t_idx = _wrap16(np.arange(H1L_ROWS, dtype=np.int16))
    return L, H, streams, scat, h1t_idx, map_a, map_b


# ---------------- device program ----------------

def _build_program(L, H):
    NCH = int((L + H).sum())
    TLO, THI = int(L.sum()) * P, int(H.sum()) * P
    nc = bacc.Bacc("TRN2", target_bir_lowering=False, debug=False,
                   num_devices=8)
    x2 = nc.dram_tensor("x2", [NNP, F], dt.bfloat16, kind="ExternalInput")
    w2_d = nc.dram_tensor("w2", [P, 2, P], dt.bfloat16, kind="ExternalInput")
    idx_d = {
        'lo': nc.dram_tensor("idx_lo", [P, TLO // 16], dt.int16,
                             kind="ExternalInput"),
        'hi': nc.dram_tensor("idx_hi", [P, THI // 16], dt.int16,
                             kind="ExternalInput"),
    }
    rowm_d = nc.dram_tensor("rowm", [P, NCH], dt.float32, kind="ExternalInput")
    nvm_d = nc.dram_tensor("nvm", [P, NCH], dt.float32, kind="ExternalInput")
    scat_lo_d = nc.dram_tensor("scat_lo", [P, NSLOT * P // 16], dt.int16,
                               kind="ExternalInput")
    scat_hi_d = nc.dram_tensor("scat_hi", [P, NSLOT * P // 16], dt.int16,
                               kind="ExternalInput")
    h1t_idx_d = nc.dram_tensor("h1t_idx", [P, H1L_ROWS // 16], dt.int16,
                               kind="ExternalInput")
    scnt_d = nc.dram_tensor("scnt", [1, 4], dt.int32, kind="ExternalInput")
    sh = nc.dram_tensor("sh", [SH_ROWS, F], dt.bfloat16, addr_space="Shared")
    h1loc = nc.dram_tensor("h1loc", [H1L_ROWS, F], dt.bfloat16)
    bar_in = nc.dram_tensor("bar_in", [1, 4], dt.float32)
    bar_out = nc.dram_tensor("bar_out", [2, 4], dt.float32)
    out_d = nc.dram_tensor("out", [H1L_ROWS, F], dt.bfloat16,
                           kind="ExternalOutput")

    nchunks = [(int(L[j]), int(H[j])) for j in range(NSLOT)]
    lo_off = np.concatenate([[0], np.cumsum(L)[:-1]]).astype(int)
    hi_off = np.concatenate([[0], np.cumsum(H)[:-1]]).astype(int)
    ch_off = np.concatenate([[0], np.cumsum(L + H)[:-1]]).astype(int)

    with tile.TileContext(nc) as tc:
        with (tc.tile_pool(name="const", bufs=1) as constp,
              tc.tile_pool(name="meta", bufs=1) as metap,
              tc.tile_pool(name="zp", bufs=1) as zp,
              tc.tile_pool(name="msg_lo", bufs=3) as msglop,
              tc.tile_pool(name="msg_hi", bufs=3) as msghip,
              tc.tile_pool(name="idxp", bufs=3) as idxp,
              tc.tile_pool(name="stagep", bufs=2) as stagep,
              tc.tile_pool(name="h1tp", bufs=2) as h1tp,
              tc.tile_pool(name="spp", bufs=12) as spp,
              tc.tile_pool(name="blkp", bufs=4) as blkp,
              tc.tile_pool(name="barp", bufs=1) as barp,
              tc.tile_pool(name="psh", bufs=2, space="PSUM") as psum_h,
              tc.tile_pool(name="pst", bufs=3, space="PSUM") as psum_t,
              tc.tile_pool(name="pso", bufs=2, space="PSUM") as psum_o):

            iota_i = constp.tile([P, P], dt.int32)
            nc.gpsimd.iota(iota_i[:], pattern=[[1, P]], base=0,
                           channel_multiplier=0)
            iota_b = constp.tile([P, P], dt.bfloat16)
            nc.vector.tensor_copy(iota_b[:], iota_i[:])
            w2_sb = constp.tile([P, 2, P], dt.bfloat16)
            nc.sync.dma_start(out=w2_sb[:], in_=w2_d[:])
            rowm_sb = metap.tile([P, NCH], dt.float32)
            nc.sync.dma_start(out=rowm_sb[:], in_=rowm_d[:])
            nvm_sb = metap.tile([P, NCH], dt.float32)
            nc.sync.dma_start(out=nvm_sb[:], in_=nvm_d[:])
            scat_lo_sb = metap.tile([P, NSLOT * P // 16], dt.int16)
            nc.sync.dma_start(out=scat_lo_sb[:], in_=scat_lo_d[:])
            scat_hi_sb = metap.tile([P, NSLOT * P // 16], dt.int16)
            nc.sync.dma_start(out=scat_hi_sb[:], in_=scat_hi_d[:])
            h1t_idx_sb = metap.tile([P, H1L_ROWS // 16], dt.int16)
            nc.sync.dma_start(out=h1t_idx_sb[:], in_=h1t_idx_d[:])

            r_flo = nc.gpsimd.alloc_register("r_flo")
            r_fhi = nc.gpsimd.alloc_register("r_fhi")
            r_llo = nc.gpsimd.alloc_register("r_llo")
            r_lhi = nc.gpsimd.alloc_register("r_lhi")
            nc.gpsimd.reg_load(r_flo, scnt_d[0:1, 0:1])
            nc.gpsimd.reg_load(r_fhi, scnt_d[0:1, 1:2])
            nc.gpsimd.reg_load(r_llo, scnt_d[0:1, 2:3])
            nc.gpsimd.reg_load(r_lhi, scnt_d[0:1, 3:4])

            # zero the shared h1 exchange buffer (both cores race: fine)
            zt = zp.tile([P, 4096], dt.bfloat16)
            nc.vector.memset(zt[:], 0.0)
            ZROWS = P * 4096 // F                     # 2048 rows per store
            for r0 in range(0, SH_ROWS, ZROWS):
                r1 = min(r0 + ZROWS, SH_ROWS)
                nc.sync.dma_start(out=sh[r0:r1, :],
                                  in_=zt[:, 0:(r1 - r0) * F // P])

            def hop(src_lo_ap, src_hi_ap, second):
                slab = {'lo': (None, -1), 'hi': (None, -1)}
                islab = {'lo': (None, -1), 'hi': (None, -1)}

                def get_idx(stream, tok0, g):
                    it, s_cur = islab[stream]
                    s = tok0 // ISLAB
                    if s != s_cur:
                        T = TLO if stream == 'lo' else THI
                        off = s * ISLAB
                        gi = min(ISLAB, T - off)
                        it = idxp.tile([P, ISLAB // 16], dt.int16, tag="idx")
                        nc.sync.dma_start(
                            out=it[:, 0:gi // 16],
                            in_=idx_d[stream][:, off // 16:(off + gi) // 16])
                        islab[stream] = (it, s)
                    j0 = (tok0 % ISLAB) // 16
                    return it[:, j0:j0 + g // 16]

                def get_chunk(stream, gpos):
                    mt, s_cur = slab[stream]
                    s, jj = divmod(gpos, GS // P)
                    if s != s_cur:
                        T = TLO if stream == 'lo' else THI
                        off = s * GS
                        g = min(GS, T - off)
                        pool = msglop if stream == 'lo' else msghip
                        mt = pool.tile([P, GS // P, F], dt.bfloat16,
                                       tag="m" + stream)
                        nc.gpsimd.dma_gather(
                            out_ap=mt[:, 0:g // P, :],
                            in_ap=src_lo_ap if stream == 'lo' else src_hi_ap,
                            idxs_ap=get_idx(stream, off, g),
                            num_idxs=g, num_idxs_reg=g,
                            elem_size=F, single_packet=False)
                        slab[stream] = (mt, s)
                    return mt, jj

                stg = None
                h1t_tile = None
                for j in range(NSLOT):
                    Lj, Hj = nchunks[j]
                    CPB = Lj + Hj
                    c0 = int(ch_off[j])
                    bi, jj = divmod(j, SCB)
                    nb_in_b = SCB if bi < NBATCH - 1 else LASTB
                    if not second:
                        hp = psum_h.tile([P, F], dt.float32, tag="hp")
                    else:
                        t01 = psum_t.tile([P, 2, P], dt.float32, tag="t01")
                        if jj == 0:
                            nidx = nb_in_b * P
                            h1t_tile = h1tp.tile([P, 2, nidx],
                                                 dt.bfloat16, tag="h1t")
                            o0 = bi * SCB * P // 16
                            nc.gpsimd.dma_gather(
                                out_ap=h1t_tile[:],
                                in_ap=h1loc[:],
                                idxs_ap=h1t_idx_sb[:, o0:o0 + nidx // 16],
                                num_idxs=nidx, num_idxs_reg=nidx,
                                elem_size=F, transpose=True,
                                single_packet=False)
                    for c in range(CPB):
                        if c < Lj:
                            mt, cj = get_chunk('lo', int(lo_off[j]) + c)
                        else:
                            mt, cj = get_chunk('hi', int(hi_off[j]) + c - Lj)
                        sp = spp.tile([P, P], dt.bfloat16, tag="sp")
                        nc.vector.tensor_scalar(
                            sp[:], iota_b[:],
                            rowm_sb[:, c0 + c:c0 + c + 1],
                            nvm_sb[:, c0 + c:c0 + c + 1],
                            mybir.AluOpType.is_equal, mybir.AluOpType.mult)
                        if not second:
                            nc.tensor.matmul(hp[:], sp[:], mt[:, cj, :],
                                             start=(c == 0),
                                             stop=(c == CPB - 1))
                        else:
                            # one accumulation group per bank: start only on
                            # the first matmul, stop only on the last (PSUM
                            # zero regions are bank-wide)
                            nc.tensor.matmul(t01[:, 0, :], mt[:, cj, 0:P],
                                             sp[:], start=(c == 0),
                                             stop=False)
                            nc.tensor.matmul(t01[:, 1, :], mt[:, cj, P:F],
                                             sp[:], start=False,
                                             stop=(c == CPB - 1))
                    if not second:
                        # h1 block -> staging; flush batch to sh + h1loc
                        if jj == 0:
                            stg = stagep.tile([P, SCB, F], dt.bfloat16,
                                              tag="stg")
                        nc.scalar.copy(stg[:, jj, :], hp[:])
                        nc.sync.dma_start(
                            out=h1loc[j * P:(j + 1) * P, :],
                            in_=stg[:, jj, :])
                        if jj == nb_in_b - 1:
                            nb = nb_in_b * P
                            o0 = bi * SCB * P // 16
                            reg_lo = r_flo if bi < NBATCH - 1 else r_llo
                            reg_hi = r_fhi if bi < NBATCH - 1 else r_lhi
                            nc.gpsimd.dma_scatter_add(
                                out_ap=sh[0:LO_LIMIT, :],
                                in_ap=stg[:, 0:nb_in_b, :],
                                idxs_ap=scat_lo_sb[:, o0:o0 + nb // 16],
                                num_idxs=nb, num_idxs_reg=reg_lo,
                                elem_size=F, single_packet=False)
                            nc.gpsimd.dma_scatter_add(
                                out_ap=sh[HI_BASE:SH_ROWS, :],
                                in_ap=stg[:, 0:nb_in_b, :],
                                idxs_ap=scat_hi_sb[:, o0:o0 + nb // 16],
                                num_idxs=nb, num_idxs_reg=reg_hi,
                                elem_size=F, single_packet=False)
                    else:
                        t0s = blkp.tile([P, P], dt.bfloat16, tag="t0s")
                        nc.scalar.copy(t0s[:], t01[:, 0, :])
                        t1s = blkp.tile([P, P], dt.bfloat16, tag="t1s")
                        nc.scalar.copy(t1s[:], t01[:, 1, :])
                        op = psum_o.tile([P, F], dt.float32, tag="op")
                        nc.tensor.matmul(op[:, 0:P], t0s[:], w2_sb[:, 1, :],
                                         start=True, stop=False)
                        nc.tensor.matmul(
                            op[:, 0:P], h1t_tile[:, 0, jj * P:(jj + 1) * P],
                            w2_sb[:, 0, :], start=False, stop=False)
                        nc.tensor.matmul(op[:, P:F], t1s[:], w2_sb[:, 1, :],
                                         start=False, stop=False)
                        nc.tensor.matmul(
                            op[:, P:F], h1t_tile[:, 1, jj * P:(jj + 1) * P],
                            w2_sb[:, 0, :], start=False, stop=True)
                        ob = blkp.tile([P, F], dt.bfloat16, tag="ob")
                        nc.scalar.copy(ob[:], op[:])
                        nc.sync.dma_start(out=out_d[j * P:(j + 1) * P, :],
                                          in_=ob[:])

            hop(x2[0:LO_LIMIT, :], x2[HI_BASE:NNP, :], second=False)

            # ---- barrier: my scatters visible -> wait for partner ----
            probe = barp.tile([2, 2], dt.bfloat16)
            nc.sync.dma_start(out=probe[0:1, :], in_=sh[0:1, 0:2])
            nc.sync.dma_start(out=probe[1:2, :], in_=sh[NNP - 1:NNP, 0:2])
            probef = barp.tile([2, 2], dt.float32)
            nc.vector.tensor_copy(probef[:], probe[:])
            nc.sync.dma_start(out=bar_in[0:1, 0:1], in_=probef[0:1, 0:1])
            nc.gpsimd.collective_compute(
                "AllGather", mybir.AluOpType.bypass,
                replica_groups=[[0, 1], [2, 3], [4, 5], [6, 7]],
                ins=[bar_in[:].opt()], outs=[bar_out[:].opt()],
            )
            barv = barp.tile([1, 4], dt.float32)
            nc.sync.dma_start(out=barv[:], in_=bar_out[0:1, :])
            barv16 = barp.tile([1, 4], dt.bfloat16)
            nc.vector.tensor_copy(barv16[:], barv[:])
            nc.sync.dma_start(out=sh[SH_ROWS - 1:SH_ROWS, 0:4], in_=barv16[:])

            hop(sh[0:SH_ROWS, :], sh[HI_BASE:SH_ROWS, :], second=True)

    nc.compile()
    return nc


# ---------------- entry point ----------------

def kernel(x, edge_index, edge_vals, W_f, W_b, bias):
    x = np.asarray(x, dtype=np.float32)
    edge_index = np.asarray(edge_index)
    edge_vals = np.asarray(edge_vals, dtype=np.float32)
    W_f = np.asarray(W_f, dtype=np.float32)
    W_b = np.asarray(W_b, dtype=np.float32)
    bias = np.asarray(bias, dtype=np.float32)

    ckey = hash((edge_index.tobytes(), edge_vals.tobytes()))
    if ckey not in _prep_cache:
        _prep_cache.clear()
        _prep_cache[ckey] = _prep(edge_index, edge_vals)
    L, H, streams, scat, h1t_idx, map_a, map_b = _prep_cache[ckey]

    pkey = (L.tobytes(), H.tobytes())
    if pkey not in _prog_cache:
        _prog_cache.clear()
        _prog_cache[pkey] = _build_program(L, H)
    nc = _prog_cache[pkey]

    full = SCB * P
    last = LASTB * P
    scnt = {0: np.array([[full, 0, last, 0]], np.int32),
            1: np.array([[0, full, 0, last]], np.int32)}

    in_maps = []
    for core in range(8):
        q, d, s = core >> 2, (core >> 1) & 1, core & 1
        st = streams[2 * d + s]
        Wd = W_f if d == 0 else W_b
        x2 = np.zeros((NNP, F), bf16)
        xq = np.transpose(x[4 * q:4 * q + 4], (1, 0, 2)).reshape(N_NODES, F)
        x2[:N_NODES] = xq.astype(bf16)
        w2 = np.zeros((P, 2, P), bf16)
        for k in range(2):
            w2[:C, k, :C] = Wd[k].astype(bf16)
            w2[C:, k, C:] = Wd[k].astype(bf16)
        in_maps.append({
            "x2": x2, "w2": w2,
            "idx_lo": _wrap16(st[0]), "idx_hi": _wrap16(st[1]),
            "rowm": st[2], "nvm": st[3],
            "scat_lo": scat[s][0], "scat_hi": scat[s][1],
            "h1t_idx": h1t_idx, "scnt": scnt[s],
        })

    results = run_bass_kernel_spmd(nc, in_maps, list(range(8))).results

    out = np.zeros((2, NNP, F), np.float32)
    for core in range(8):
        q, s = core >> 2, core & 1
        oc = results[core]["out"].astype(np.float32)
        blocks = map_a if s == 0 else map_b
        for j, b in enumerate(blocks):
            out[q, b * P:(b + 1) * P] += oc[j * P:(j + 1) * P]

    res = np.empty((B, N_NODES, C), np.float32)
    for q in range(2):
        for bi in range(4):
            res[4 * q + bi] = out[q, :N_NODES, bi * C:(bi + 1) * C]
    res += bias.reshape(1, 1, C)
    return res


# revision 3
# speedup vs baseline: 1.0629x; 1.0629x over previous
"""DiffusionGraphConv on 8 Trainium2 NeuronCores (Bass/Tile), v2.

out = sum_k (D^-1 A)^k x W_f[k] + ((D^-1 A)^T)^k x W_b[k] + bias, K=2,
N=50000 nodes, E=800000 edges, B=8, C_in=C_out=64, f32.

Sharding: 8 cores = 2 batch-quads x 2 directions x 2 dst-halves.
Tokens are 512B (4 batches x 64 feats, bf16) so each DMA-gather descriptor
runs at full bus bandwidth.  The pair (2k, 2k+1) = one (quad, direction)
job split by destination blocks (half A: blocks 0..195, half B: 196..390);
h1 = A@h0 is exchanged through a pair-shared HBM tensor: scatter-add into
a zeroed buffer with data-driven row indices keeps the SPMD instruction
stream identical across cores (each scatter call is entirely-valid or
entirely-dead per core; num_idxs_reg comes from input data via reg_load).
A tiny pairwise AllGather is the inter-hop barrier.

Per hop, per dst slot: one-hot scatter matrices S'[t,r] are built by one
DVE tensor_scalar (bf16) and the sparse accumulate is a PE matmul per
128-edge chunk into PSUM.  Hop 2 computes h2 transposed (gathered chunk as
the stationary operand) so the output weight matmuls need no PE
transposes; the W0 term reuses h1 via a transpose-mode dma_gather from a
local slot-major copy.
"""
import numpy as np
import ml_dtypes

import concourse.bacc as bacc
import concourse.tile as tile
import concourse.mybir as mybir
from concourse.bass_utils import run_bass_kernel_spmd

P = 128
N_NODES = 50000
N_EDGES = 800000
B, C = 8, 64
F = 256                  # feats per token: 4 batches x 64, bf16 -> 512B
NNP = 50048              # nodes padded to a multiple of 128
NB = NNP // P            # 391 blocks
NSLOT = 196              # slots per half (A: blocks 0..195, B: 196..390+dummy)
NA = 196                 # blocks in half A
LO_LIMIT = 32768         # lo gather window [0, 32768)
HI_BASE = NNP - 32768    # hi window [17280, NNP)
SH_ROWS = NNP + P        # shared h1 + pad rows (barrier dep marker)
H1L_ROWS = NSLOT * P     # local slot-major h1 copy
GS = 2048                # gather slab tokens
ISLAB = 8192             # idx slab tokens (4 gather slabs per idx dma)
SCB = 16                 # slots per scatter/staging batch
NBATCH = (NSLOT + SCB - 1) // SCB
LASTB = NSLOT - (NBATCH - 1) * SCB      # slots in the last batch (4)
bf16 = ml_dtypes.bfloat16
dt = mybir.dt

_prog_cache = {}
_prep_cache = {}


# ---------------- host-side prep ----------------

def _variant_counts(dst, src, blocks):
    """(must_lo, must_hi, flex, tot) per slot for one (direction, half)."""
    blk = dst >> 7
    inv = np.full(NB, -1, np.int64)
    inv[blocks] = np.arange(len(blocks))
    m = inv[blk] >= 0
    slot = inv[blk[m]]
    s = src[m]
    must_lo = np.bincount(slot[s < HI_BASE], minlength=NSLOT)
    must_hi = np.bincount(slot[s >= LO_LIMIT], minlength=NSLOT)
    flex = np.bincount(slot[(s >= HI_BASE) & (s < LO_LIMIT)], minlength=NSLOT)
    return must_lo, must_hi, flex, must_lo + must_hi + flex


def _choose_chunks4(cnts):
    """Shared per-slot (L, H) minimizing L+H over the 4 stream variants."""
    L = np.zeros(NSLOT, np.int64)
    H = np.zeros(NSLOT, np.int64)
    f2l = [np.zeros(NSLOT, np.int64) for _ in cnts]
    for j in range(NSLOT):
        lmin = max((ml[j] + P - 1) // P for ml, mh, fx, tot in cnts)
        best = None
        for Lc in range(lmin, lmin + 4):
            Hc = 0
            for ml, mh, fx, tot in cnts:
                lo = min(Lc * P, ml[j] + fx[j])
                Hc = max(Hc, (max(mh[j], tot[j] - lo) + P - 1) // P)
            if best is None or Lc + Hc < best[0] + best[1]:
                best = (Lc, Hc)
        L[j], H[j] = best
        for v, (ml, mh, fx, tot) in enumerate(cnts):
            lo = min(L[j] * P, ml[j] + fx[j])
            lo = max(lo, tot[j] - H[j] * P)   # ensure hi fits
            f2l[v][j] = lo - ml[j]
    return L, H, f2l


def _build_stream(dst, src, nv, blocks, L, H, flex_to_lo):
    """Padded lo/hi token streams + chunk-major meta for one variant."""
    lo_chunk_off = np.concatenate([[0], np.cumsum(L)[:-1]])
    hi_chunk_off = np.concatenate([[0], np.cumsum(H)[:-1]])
    chunk_off = np.concatenate([[0], np.cumsum(L + H)[:-1]])
    NCH = int((L + H).sum())
    TLO, THI = int(L.sum()) * P, int(H.sum()) * P

    blk = (dst >> 7).astype(np.int64)
    inv = np.full(NB, -1, np.int64)
    inv[blocks] = np.arange(len(blocks))
    m = inv[blk] >= 0
    d_s, s_s, nv_s = dst[m], src[m], nv[m]
    slot = inv[blk[m]]

    lo = s_s < HI_BASE
    flex = (s_s >= HI_BASE) & (s_s < LO_LIMIT)
    fidx = np.flatnonzero(flex)
    forder = np.argsort(slot[fidx], kind="stable")
    fslot = slot[fidx[forder]]
    fcnt = np.bincount(fslot, minlength=NSLOT)
    fstart = np.concatenate([[0], np.cumsum(fcnt)[:-1]])
    frank = np.arange(fidx.size) - fstart[fslot]
    lo = lo.copy()
    lo[fidx[forder]] = frank < flex_to_lo[fslot]
    assert (np.bincount(slot[lo], minlength=NSLOT) <= L * P).all()
    assert (np.bincount(slot[~lo], minlength=NSLOT) <= H * P).all()

    order = np.lexsort((~lo, slot))
    d_s, s_s, nv_s = d_s[order], s_s[order], nv_s[order]
    slot_s, lo_s = slot[order], lo[order]
    gid = slot_s * 2 + (~lo_s).astype(np.int64)
    cnt = np.bincount(gid, minlength=NSLOT * 2)
    gstart = np.concatenate([[0], np.cumsum(cnt)[:-1]])
    rank = np.arange(d_s.size) - gstart[gid]
    pos = np.where(lo_s, lo_chunk_off[slot_s] * P + rank,
                   hi_chunk_off[slot_s] * P + rank)

    idx_lo = np.zeros(TLO, np.int16)
    nv_lo = np.zeros(TLO, np.float32)
    rm_lo = np.zeros(TLO, np.float32)
    idx_hi = np.zeros(THI, np.int16)
    nv_hi = np.zeros(THI, np.float32)
    rm_hi = np.zeros(THI, np.float32)
    mm = lo_s
    idx_lo[pos[mm]] = s_s[mm].astype(np.int16)
    nv_lo[pos[mm]] = nv_s[mm]
    rm_lo[pos[mm]] = (d_s[mm] & 127).astype(np.float32)
    mm = ~lo_s
    idx_hi[pos[mm]] = (s_s[mm] - HI_BASE).astype(np.int16)
    nv_hi[pos[mm]] = nv_s[mm]
    rm_hi[pos[mm]] = (d_s[mm] & 127).astype(np.float32)

    rowm = np.zeros((P, NCH), np.float32)
    nvm = np.zeros((P, NCH), np.float32)
    lo_cols = np.concatenate(
        [chunk_off[j] + np.arange(L[j]) for j in range(NSLOT)])
    hi_cols = np.concatenate(
        [chunk_off[j] + L[j] + np.arange(H[j]) for j in range(NSLOT)])
    if TLO:
        rowm[:, lo_cols] = rm_lo.reshape(-1, P).T
        nvm[:, lo_cols] = nv_lo.reshape(-1, P).T
    if THI:
        rowm[:, hi_cols] = rm_hi.reshape(-1, P).T
        nvm[:, hi_cols] = nv_hi.reshape(-1, P).T
    return idx_lo, idx_hi, rowm, nvm


def _wrap16(a):
    """[T] -> [128, T/16]; token i at [i%16, i//16], replicated 8x."""
    return np.ascontiguousarray(np.tile(a.reshape(-1, 16).T, (8, 1)))


def _scatter_idx(blocks):
    """Scatter row idx streams, split by half OWNERSHIP (pure per core):
    half A rows (< 25088) go to the lo call, half B rows to the hi call."""
    rows = np.full(NSLOT * P, -1, np.int64)
    for j, b in enumerate(blocks):
        rows[j * P:(j + 1) * P] = b * P + np.arange(P)
    dummy = slice(len(blocks) * P, NSLOT * P)
    rows[dummy] = NNP - 1   # always-written row; dummy h1 value is 0
    lo = np.where((rows >= 0) & (rows < NA * P), rows, -1).astype(np.int16)
    hi = np.where(rows >= NA * P, rows - HI_BASE, -1).astype(np.int16)
    return _wrap16(lo), _wrap16(hi)


def _prep(edge_index, edge_vals):
    rows = edge_index[0].astype(np.int64)
    cols = edge_index[1].astype(np.int64)
    deg = np.zeros(N_NODES, np.float32)
    np.add.at(deg, rows, edge_vals)
    deg += np.float32(1e-8)
    nv = (edge_vals / deg[rows]).astype(np.float32)

    # half split; per-variant rank-sorted slot maps (slot j = j-th largest
    # count block of that variant's half) minimize the per-slot max
    cntf = np.bincount(rows >> 7, minlength=NB)  # fwd: dst=rows
    cntb = np.bincount(cols >> 7, minlength=NB)  # bwd: dst=cols
    blocks_a = np.arange(0, NA)
    blocks_b = np.arange(NA, NB)
    maps = [blocks_a[np.argsort(-cntf[blocks_a], kind="stable")],
            blocks_b[np.argsort(-cntf[blocks_b], kind="stable")],
            blocks_a[np.argsort(-cntb[blocks_a], kind="stable")],
            blocks_b[np.argsort(-cntb[blocks_b], kind="stable")]]

    variants = [(rows, cols, maps[0]), (rows, cols, maps[1]),
                (cols, rows, maps[2]), (cols, rows, maps[3])]
    cnts = [_variant_counts(d, s, m) for d, s, m in variants]
    L, H, f2l = _choose_chunks4(cnts)
    streams = [_build_stream(d, s, nv, m, L, H, f2l[v])
               for v, (d, s, m) in enumerate(variants)]
    scat = [_scatter_idx(m) for m in maps]
    h1t_idx = _wrap16(np.arange(H1L_ROWS, dtype=np.int16))
    return L, H, streams, scat, h1t_idx, maps


# ---------------- device program ----------------

def _build_program(L, H):
    NCH = int((L + H).sum())
    TLO, THI = int(L.sum()) * P, int(H.sum()) * P
    nc = bacc.Bacc("TRN2", target_bir_lowering=False, debug=False,
                   num_devices=8)
    x2 = nc.dram_tensor("x2", [NNP, F], dt.bfloat16, kind="ExternalInput")
    w2_d = nc.dram_tensor("w2", [P, 2, P], dt.bfloat16, kind="ExternalInput")
    idx_d = {
        'lo': nc.dram_tensor("idx_lo", [P, TLO // 16], dt.int16,
                             kind="ExternalInput"),
        'hi': nc.dram_tensor("idx_hi", [P, THI // 16], dt.int16,
                             kind="ExternalInput"),
    }
    rowm_d = nc.dram_tensor("rowm", [P, NCH], dt.float32, kind="ExternalInput")
    nvm_d = nc.dram_tensor("nvm", [P, NCH], dt.float32, kind="ExternalInput")
    scat_lo_d = nc.dram_tensor("scat_lo", [P, NSLOT * P // 16], dt.int16,
                               kind="ExternalInput")
    scat_hi_d = nc.dram_tensor("scat_hi", [P, NSLOT * P // 16], dt.int16,
                               kind="ExternalInput")
    h1t_idx_d = nc.dram_tensor("h1t_idx", [P, H1L_ROWS // 16], dt.int16,
                               kind="ExternalInput")
    scnt_d = nc.dram_tensor("scnt", [1, 4], dt.int32, kind="ExternalInput")
    sh = nc.dram_tensor("sh", [SH_ROWS, F], dt.bfloat16, addr_space="Shared")
    h1loc = nc.dram_tensor("h1loc", [H1L_ROWS, F], dt.bfloat16)
    bar_in = nc.dram_tensor("bar_in", [1, 4], dt.float32)
    bar_out = nc.dram_tensor("bar_out", [2, 4], dt.float32)
    out_d = nc.dram_tensor("out", [H1L_ROWS, F], dt.bfloat16,
                           kind="ExternalOutput")

    nchunks = [(int(L[j]), int(H[j])) for j in range(NSLOT)]
    lo_off = np.concatenate([[0], np.cumsum(L)[:-1]]).astype(int)
    hi_off = np.concatenate([[0], np.cumsum(H)[:-1]]).astype(int)
    ch_off = np.concatenate([[0], np.cumsum(L + H)[:-1]]).astype(int)

    with tile.TileContext(nc) as tc:
        with (tc.tile_pool(name="const", bufs=1) as constp,
              tc.tile_pool(name="meta", bufs=1) as metap,
              tc.tile_pool(name="zp", bufs=1) as zp,
              tc.tile_pool(name="msg_lo", bufs=6) as msglop,
              tc.tile_pool(name="msg_hi", bufs=6) as msghip,
              tc.tile_pool(name="idxp", bufs=6) as idxp,
              tc.tile_pool(name="stagep", bufs=3) as stagep,
              tc.tile_pool(name="h1tp", bufs=3) as h1tp,
              tc.tile_pool(name="spp", bufs=24) as spp,
              tc.tile_pool(name="blkp", bufs=6) as blkp,
              tc.tile_pool(name="barp", bufs=1) as barp,
              tc.tile_pool(name="psh", bufs=3, space="PSUM") as psum_h,
              tc.tile_pool(name="pst", bufs=3, space="PSUM") as psum_t,
              tc.tile_pool(name="pso", bufs=2, space="PSUM") as psum_o):

            iota_i = constp.tile([P, P], dt.int32)
            nc.gpsimd.iota(iota_i[:], pattern=[[1, P]], base=0,
                           channel_multiplier=0)
            iota_b = constp.tile([P, P], dt.bfloat16)
            nc.vector.tensor_copy(iota_b[:], iota_i[:])
            w2_sb = constp.tile([P, 2, P], dt.bfloat16)
            nc.sync.dma_start(out=w2_sb[:], in_=w2_d[:])
            rowm_sb = metap.tile([P, NCH], dt.float32)
            nc.sync.dma_start(out=rowm_sb[:], in_=rowm_d[:])
            nvm_sb = metap.tile([P, NCH], dt.float32)
            nc.sync.dma_start(out=nvm_sb[:], in_=nvm_d[:])
            scat_lo_sb = metap.tile([P, NSLOT * P // 16], dt.int16)
            nc.sync.dma_start(out=scat_lo_sb[:], in_=scat_lo_d[:])
            scat_hi_sb = metap.tile([P, NSLOT * P // 16], dt.int16)
            nc.sync.dma_start(out=scat_hi_sb[:], in_=scat_hi_d[:])
            h1t_idx_sb = metap.tile([P, H1L_ROWS // 16], dt.int16)
            nc.sync.dma_start(out=h1t_idx_sb[:], in_=h1t_idx_d[:])

            r_flo = nc.gpsimd.alloc_register("r_flo")
            r_fhi = nc.gpsimd.alloc_register("r_fhi")
            r_llo = nc.gpsimd.alloc_register("r_llo")
            r_lhi = nc.gpsimd.alloc_register("r_lhi")
            nc.gpsimd.reg_load(r_flo, scnt_d[0:1, 0:1])
            nc.gpsimd.reg_load(r_fhi, scnt_d[0:1, 1:2])
            nc.gpsimd.reg_load(r_llo, scnt_d[0:1, 2:3])
            nc.gpsimd.reg_load(r_lhi, scnt_d[0:1, 3:4])

            # zero the shared h1 exchange buffer (both cores race: fine)
            zt = zp.tile([P, 4096], dt.bfloat16)
            nc.vector.memset(zt[:], 0.0)
            ZROWS = P * 4096 // F                     # 2048 rows per store
            for r0 in range(0, SH_ROWS, ZROWS):
                r1 = min(r0 + ZROWS, SH_ROWS)
                nc.sync.dma_start(out=sh[r0:r1, :],
                                  in_=zt[:, 0:(r1 - r0) * F // P])

            def hop(src_lo_ap, src_hi_ap, second):
                slab = {'lo': (None, -1), 'hi': (None, -1)}
                islab = {'lo': (None, -1), 'hi': (None, -1)}

                def get_idx(stream, tok0, g):
                    it, s_cur = islab[stream]
                    s = tok0 // ISLAB
                    if s != s_cur:
                        T = TLO if stream == 'lo' else THI
                        off = s * ISLAB
                        gi = min(ISLAB, T - off)
                        it = idxp.tile([P, ISLAB // 16], dt.int16, tag="idx")
                        nc.sync.dma_start(
                            out=it[:, 0:gi // 16],
                            in_=idx_d[stream][:, off // 16:(off + gi) // 16])
                        islab[stream] = (it, s)
                    j0 = (tok0 % ISLAB) // 16
                    return it[:, j0:j0 + g // 16]

                def get_chunk(stream, gpos):
                    mt, s_cur = slab[stream]
                    s, jj = divmod(gpos, GS // P)
                    if s != s_cur:
                        T = TLO if stream == 'lo' else THI
                        off = s * GS
                        g = min(GS, T - off)
                        pool = msglop if stream == 'lo' else msghip
                        mt = pool.tile([P, GS // P, F], dt.bfloat16,
                                       tag="m" + stream)
                        nc.gpsimd.dma_gather(
                            out_ap=mt[:, 0:g // P, :],
                            in_ap=src_lo_ap if stream == 'lo' else src_hi_ap,
                            idxs_ap=get_idx(stream, off, g),
                            num_idxs=g, num_idxs_reg=g,
                            elem_size=F, single_packet=False)
                        slab[stream] = (mt, s)
                    return mt, jj

                stg = None
                h1t_tile = None
                for j in range(NSLOT):
                    Lj, Hj = nchunks[j]
                    CPB = Lj + Hj
                    c0 = int(ch_off[j])
                    bi, jj = divmod(j, SCB)
                    nb_in_b = SCB if bi < NBATCH - 1 else LASTB
                    if not second:
                        hp = psum_h.tile([P, F], dt.float32, tag="hp")
                    else:
                        t01 = psum_t.tile([P, 2, P], dt.float32, tag="t01")
                        if jj == 0:
                            nidx = nb_in_b * P
                            h1t_tile = h1tp.tile([P, 2, nidx],
                                                 dt.bfloat16, tag="h1t")
                            o0 = bi * SCB * P // 16
                            nc.gpsimd.dma_gather(
                                out_ap=h1t_tile[:],
                                in_ap=h1loc[:],
                                idxs_ap=h1t_idx_sb[:, o0:o0 + nidx // 16],
                                num_idxs=nidx, num_idxs_reg=nidx,
                                elem_size=F, transpose=True,
                                single_packet=False)
                    for c in range(CPB):
                        if c < Lj:
                            mt, cj = get_chunk('lo', int(lo_off[j]) + c)
                        else:
                            mt, cj = get_chunk('hi', int(hi_off[j]) + c - Lj)
                        sp = spp.tile([P, P], dt.bfloat16, tag="sp")
                        nc.vector.tensor_scalar(
                            sp[:], iota_b[:],
                            rowm_sb[:, c0 + c:c0 + c + 1],
                            nvm_sb[:, c0 + c:c0 + c + 1],
                            mybir.AluOpType.is_equal, mybir.AluOpType.mult)
                        if not second:
                            nc.tensor.matmul(hp[:], sp[:], mt[:, cj, :],
                                             start=(c == 0),
                                             stop=(c == CPB - 1))
                        else:
                            # one accumulation group per bank: start only on
                            # the first matmul, stop only on the last (PSUM
                            # zero regions are bank-wide)
                            nc.tensor.matmul(t01[:, 0, :], mt[:, cj, 0:P],
                                             sp[:], start=(c == 0),
                                             stop=False)
                            nc.tensor.matmul(t01[:, 1, :], mt[:, cj, P:F],
                                             sp[:], start=False,
                                             stop=(c == CPB - 1))
                    if not second:
                        # h1 block -> staging; flush batch to sh + h1loc
                        if jj == 0:
                            stg = stagep.tile([P, SCB, F], dt.bfloat16,
                                              tag="stg")
                        nc.scalar.copy(stg[:, jj, :], hp[:])
                        nc.sync.dma_start(
                            out=h1loc[j * P:(j + 1) * P, :],
                            in_=stg[:, jj, :])
                        if jj == nb_in_b - 1:
                            nb = nb_in_b * P
                            o0 = bi * SCB * P // 16
                            reg_lo = r_flo if bi < NBATCH - 1 else r_llo
                            reg_hi = r_fhi if bi < NBATCH - 1 else r_lhi
                            nc.gpsimd.dma_scatter_add(
                                out_ap=sh[0:LO_LIMIT, :],
                                in_ap=stg[:, 0:nb_in_b, :],
                                idxs_ap=scat_lo_sb[:, o0:o0 + nb // 16],
                                num_idxs=nb, num_idxs_reg=reg_lo,
                                elem_size=F, single_packet=False)
                            nc.gpsimd.dma_scatter_add(
                                out_ap=sh[HI_BASE:SH_ROWS, :],
                                in_ap=stg[:, 0:nb_in_b, :],
                                idxs_ap=scat_hi_sb[:, o0:o0 + nb // 16],
                                num_idxs=nb, num_idxs_reg=reg_hi,
                                elem_size=F, single_packet=False)
                    else:
                        t0s = blkp.tile([P, P], dt.bfloat16, tag="t0s")
                        nc.scalar.copy(t0s[:], t01[:, 0, :])
                        t1s = blkp.tile([P, P], dt.bfloat16, tag="t1s")
                        nc.scalar.copy(t1s[:], t01[:, 1, :])
                        op = psum_o.tile([P, F], dt.float32, tag="op")
                        nc.tensor.matmul(op[:, 0:P], t0s[:], w2_sb[:, 1, :],
                                         start=True, stop=False)
                        nc.tensor.matmul(
                            op[:, 0:P], h1t_tile[:, 0, jj * P:(jj + 1) * P],
                            w2_sb[:, 0, :], start=False, stop=False)
                        nc.tensor.matmul(op[:, P:F], t1s[:], w2_sb[:, 1, :],
                                         start=False, stop=False)
                        nc.tensor.matmul(
                            op[:, P:F], h1t_tile[:, 1, jj * P:(jj + 1) * P],
                            w2_sb[:, 0, :], start=False, stop=True)
                        ob = blkp.tile([P, F], dt.bfloat16, tag="ob")
                        nc.scalar.copy(ob[:], op[:])
                        nc.sync.dma_start(out=out_d[j * P:(j + 1) * P, :],
                                          in_=ob[:])

            hop(x2[0:LO_LIMIT, :], x2[HI_BASE:NNP, :], second=False)

            # ---- barrier: my scatters visible -> wait for partner ----
            probe = barp.tile([2, 2], dt.bfloat16)
            nc.sync.dma_start(out=probe[0:1, :], in_=sh[0:1, 0:2])
            nc.sync.dma_start(out=probe[1:2, :], in_=sh[NNP - 1:NNP, 0:2])
            probef = barp.tile([2, 2], dt.float32)
            nc.vector.tensor_copy(probef[:], probe[:])
            nc.sync.dma_start(out=bar_in[0:1, 0:1], in_=probef[0:1, 0:1])
            nc.gpsimd.collective_compute(
                "AllGather", mybir.AluOpType.bypass,
                replica_groups=[[0, 1], [2, 3], [4, 5], [6, 7]],
                ins=[bar_in[:].opt()], outs=[bar_out[:].opt()],
            )
            barv = barp.tile([1, 4], dt.float32)
            nc.sync.dma_start(out=barv[:], in_=bar_out[0:1, :])
            barv16 = barp.tile([1, 4], dt.bfloat16)
            nc.vector.tensor_copy(barv16[:], barv[:])
            nc.sync.dma_start(out=sh[SH_ROWS - 1:SH_ROWS, 0:4], in_=barv16[:])

            hop(sh[0:SH_ROWS, :], sh[HI_BASE:SH_ROWS, :], second=True)

    nc.compile()
    return nc


# ---------------- entry point ----------------

def kernel(x, edge_index, edge_vals, W_f, W_b, bias):
    x = np.asarray(x, dtype=np.float32)
    edge_index = np.asarray(edge_index)
    edge_vals = np.asarray(edge_vals, dtype=np.float32)
    W_f = np.asarray(W_f, dtype=np.float32)
    W_b = np.asarray(W_b, dtype=np.float32)
    bias = np.asarray(bias, dtype=np.float32)

    ckey = hash((edge_index.tobytes(), edge_vals.tobytes()))
    if ckey not in _prep_cache:
        _prep_cache.clear()
        _prep_cache[ckey] = _prep(edge_index, edge_vals)
    L, H, streams, scat, h1t_idx, maps = _prep_cache[ckey]

    pkey = (L.tobytes(), H.tobytes())
    if pkey not in _prog_cache:
        _prog_cache.clear()
        _prog_cache[pkey] = _build_program(L, H)
    nc = _prog_cache[pkey]

    full = SCB * P
    last = LASTB * P
    scnt = {0: np.array([[full, 0, last, 0]], np.int32),
            1: np.array([[0, full, 0, last]], np.int32)}

    in_maps = []
    for core in range(8):
        q, d, s = core >> 2, (core >> 1) & 1, core & 1
        st = streams[2 * d + s]
        Wd = W_f if d == 0 else W_b
        x2 = np.zeros((NNP, F), bf16)
        xq = np.transpose(x[4 * q:4 * q + 4], (1, 0, 2)).reshape(N_NODES, F)
        x2[:N_NODES] = xq.astype(bf16)
        w2 = np.zeros((P, 2, P), bf16)
        for k in range(2):
            w2[:C, k, :C] = Wd[k].astype(bf16)
            w2[C:, k, C:] = Wd[k].astype(bf16)
        in_maps.append({
            "x2": x2, "w2": w2,
            "idx_lo": _wrap16(st[0]), "idx_hi": _wrap16(st[1]),
            "rowm": st[2], "nvm": st[3],
            "scat_lo": scat[2 * d + s][0], "scat_hi": scat[2 * d + s][1],
            "h1t_idx": h1t_idx, "scnt": scnt[s],
        })

    results = run_bass_kernel_spmd(nc, in_maps, list(range(8))).results

    out = np.zeros((2, NNP, F), np.float32)
    for core in range(8):
        q, d, s = core >> 2, (core >> 1) & 1, core & 1
        oc = results[core]["out"].astype(np.float32)
        for j, b in enumerate(maps[2 * d + s]):
            out[q, b * P:(b + 1) * P] += oc[j * P:(j + 1) * P]

    res = np.empty((B, N_NODES, C), np.float32)
    for q in range(2):
        for bi in range(4):
            res[4 * q + bi] = out[q, :N_NODES, bi * C:(bi + 1) * C]
    res += bias.reshape(1, 1, C)
    return res
